# revision 35
# baseline (speedup 1.0000x reference)
"""Multi-head attention (per-head full-dim projections) on 8 TRN2 NeuronCores.

Problem: B=16, N=1024, D=512, H=8
  k_/v_/q_ = x @ W{k,v,q}[h].T + b  -> per-head [B,N,D]
  attn = softmax((q_ @ k_^T)/sqrt(D)); out = attn @ v_
  rep = interleave_heads(out) @ Wo.T + bo

Sharding: data parallel over batch (2 batches/core, no collectives).

Fused-projection algebra (host-precomputed, cuts device matmuls 25%):
  scores = (q Wq^T)(k Wk^T)^T/sqrt(D) + bias terms
         = q M k^T + c[j] + (i-only terms that cancel in softmax)
    with M = Wq^T Wk / sqrt(D)  and  c[b,h,j] = ((bq Wk) . k[b,j]) / sqrt(D)
    -> the k-projection disappears; c enters free as the per-partition bias
       of the exp eviction.
  attn @ (v Wv^T + bv) @ Wo_h^T = attn v G^T + bv Wo_h^T
    with G^T = Wv^T Wo_h^T  -> the v-projection disappears; the bv term is a
    constant vector folded into cv = bo + sum_h bv Wo_h^T (host).

Device per (b, h) — all matmuls contract over the partition dim, layouts
arranged on host so there are zero on-device transposes:
  tT[d2,i]  = (lhsT=M tile [d,d2c], rhs=qT [d,i])          32 MM
  S_T[j,i]  = (lhsT=kT [d2,jc],    rhs=tT [d2,i])          64 MM
  E_T = exp(S_T + c[j])  (ACT, per-partition bias; no max-subtract: scores
        ~N(0,1) so exp is safe)
  den[i]: DVE adds over j-chunks + gpsimd partition_all_reduce (no PE);
        reciprocal taken after a tiny DRAM-roundtrip reshape to [i%P, i//P]
  numT[d,i] = (lhsT=v tile [j,dc],  rhs=E_T [j,i])         64 MM
  rep[i,o] += (lhsT=numT [d,ic],    rhs=G^T [d,o])         32 MM
        per-head 1/den[i] fused into the eviction as a per-partition scalar
        (scalar_tensor_tensor: rep = psum*recip + rep; +cv at h==0)
"""

import sys

sys.path.insert(0, "/opt/trn_rl_repo")

from contextlib import ExitStack

import numpy as np
import ml_dtypes

B, N, D, H = 16, 1024, 512, 8
NCORES = 8
BPC = B // NCORES  # batches per core
P = 128
DC = D // P        # 4 feature chunks
NT = N // P        # 8 token chunks
FD = 512           # matmul moving free dim / PSUM bank
IH = N // FD       # 2 halves of the token axis

BF16 = ml_dtypes.bfloat16

_cached = {}


def _build():
    import concourse.bass as bass
    import concourse.tile as tile
    from concourse import bacc, mybir, bass_isa

    f32 = mybir.dt.float32
    bf16 = mybir.dt.bfloat16

    nc = bacc.Bacc(None, target_bir_lowering=False, debug=False)

    qT_d = nc.dram_tensor("qT", [BPC, P, DC, N], bf16, kind="ExternalInput")
    kT_d = nc.dram_tensor("kT", [BPC, P, DC, N], bf16, kind="ExternalInput")
    vN_d = nc.dram_tensor("vN", [BPC, P, NT, D], bf16, kind="ExternalInput")
    m_d = nc.dram_tensor("Mp", [H, P, DC, D], bf16, kind="ExternalInput")
    g_d = nc.dram_tensor("Gp", [H, P, DC, D], bf16, kind="ExternalInput")
    cj_d = nc.dram_tensor("cj", [BPC, H, P, NT], f32, kind="ExternalInput")
    cv_d = nc.dram_tensor("cv", [1, D], f32, kind="ExternalInput")
    out_d = nc.dram_tensor("out", [BPC, N, D], f32, kind="ExternalOutput")

    with tile.TileContext(nc) as tc, ExitStack() as ctx:
        consts = ctx.enter_context(tc.tile_pool(name="consts", bufs=1))
        acts = ctx.enter_context(tc.tile_pool(name="acts", bufs=2))
        wpool = ctx.enter_context(tc.tile_pool(name="wpool", bufs=2))
        projp = ctx.enter_context(tc.tile_pool(name="projp", bufs=2))
        etp = ctx.enter_context(tc.tile_pool(name="etp", bufs=2))
        ohp = ctx.enter_context(tc.tile_pool(name="ohp", bufs=2))
        rcp = ctx.enter_context(tc.tile_pool(name="rcp", bufs=2))
        repp = ctx.enter_context(tc.tile_pool(name="repp", bufs=1))
        cjp = ctx.enter_context(tc.tile_pool(name="cjp", bufs=2))
        mmps = ctx.enter_context(tc.tile_pool(name="mmps", bufs=4, space="PSUM"))
        repps = ctx.enter_context(tc.tile_pool(name="repps", bufs=4, space="PSUM"))
        dramp = ctx.enter_context(tc.tile_pool(name="dramp", bufs=2, space="DRAM"))

        cv_full = consts.tile([P, D], f32)
        nc.gpsimd.dma_start(out=cv_full[:], in_=cv_d[0:1, :].to_broadcast([P, D]))

        for b in range(BPC):
            def load_weights(h):
                mh = wpool.tile([P, DC, D], bf16, name="mh")
                gh = wpool.tile([P, DC, D], bf16, name="gh")
                nc.sync.dma_start(out=mh[:], in_=m_d[h])
                nc.sync.dma_start(out=gh[:], in_=g_d[h])
                return mh, gh

            qT = acts.tile([P, DC, N], bf16, name="qT_sb")
            kT = acts.tile([P, DC, N], bf16, name="kT_sb")
            vN = acts.tile([P, NT, D], bf16, name="vN_sb")
            cj_sb = cjp.tile([P, H, NT], f32, name="cj_sb")
            nc.gpsimd.dma_start(out=cj_sb[:], in_=cj_d[b].rearrange("h p t -> p h t"))
            # interleave weight-h0 and activation chunk DMAs so the first
            # matmuls (Mh dc0 + qT dc0) can start ASAP
            w0 = None
            for dc in range(DC):
                if b == 0 and dc == 0:
                    nc.sync.dma_start(out=qT[:, 0, :], in_=qT_d[b, :, 0, :])
                    w0 = load_weights(0)
                    nc.sync.dma_start(out=kT[:, 0, :], in_=kT_d[b, :, 0, :])
                else:
                    nc.sync.dma_start(out=qT[:, dc, :], in_=qT_d[b, :, dc, :])
                    nc.sync.dma_start(out=kT[:, dc, :], in_=kT_d[b, :, dc, :])
                nc.sync.dma_start(out=vN[:, 2 * dc, :], in_=vN_d[b, :, 2 * dc, :])
                nc.sync.dma_start(out=vN[:, 2 * dc + 1, :], in_=vN_d[b, :, 2 * dc + 1, :])

            rep = repp.tile([P, NT, D], f32, name="rep")

            for h in range(H):
                if b == 0 and h == 0:
                    mh, gh = w0
                else:
                    mh, gh = load_weights(h)

                # ---- tT = (q M)^T : [d2, i] ----
                tT = projp.tile([P, DC, N], bf16, name="tT")
                for ec in range(DC):
                    pq = [mmps.tile([P, FD], f32, name="mm") for _ in range(IH)]
                    for dc in range(DC):
                        for ih in range(IH):
                            nc.tensor.matmul(
                                pq[ih][:],
                                lhsT=mh[:, dc, ec * P:(ec + 1) * P],
                                rhs=qT[:, dc, ih * FD:(ih + 1) * FD],
                                start=(dc == 0),
                                stop=(dc == DC - 1),
                            )
                    for ih in range(IH):
                        nc.scalar.copy(out=tT[:, ec, ih * FD:(ih + 1) * FD], in_=pq[ih][:])

                # ---- scores S_T = (t k^T)^T + c[j]; exp; den on DVE ----
                et = etp.tile([P, NT, N], bf16, name="et")  # E_T [j, i]
                den_acc = rcp.tile([P, N], bf16, name="den_acc")
                den_red = rcp.tile([P, N], f32, name="den_red")
                for jc in range(NT):
                    ps = [mmps.tile([P, FD], f32, name="mm") for _ in range(IH)]
                    for ec in range(DC):
                        for ih in range(IH):
                            nc.tensor.matmul(
                                ps[ih][:],
                                lhsT=kT[:, ec, jc * P:(jc + 1) * P],
                                rhs=tT[:, ec, ih * FD:(ih + 1) * FD],
                                start=(ec == 0),
                                stop=(ec == DC - 1),
                            )
                    for ih in range(IH):
                        nc.scalar.activation(
                            out=et[:, jc, ih * FD:(ih + 1) * FD],
                            in_=ps[ih][:],
                            func=mybir.ActivationFunctionType.Exp,
                            bias=cj_sb[:, h, jc:jc + 1],
                        )
                    if jc == 1:
                        nc.vector.tensor_add(den_acc[:], et[:, 0, :], et[:, 1, :])
                    elif jc > 1:
                        nc.vector.tensor_add(den_acc[:], et[:, jc, :], den_acc[:])

                # ---- partition-reduce den on gpsimd, recip -> [i%P, i//P] ----
                nc.gpsimd.partition_all_reduce(
                    den_red[:], den_acc[:], P, bass_isa.ReduceOp.add
                )
                rscratch = dramp.tile([1, N], f32, name="rscratch")
                nc.gpsimd.dma_start(out=rscratch[:], in_=den_red[0:1, :])
                den_pp = rcp.tile([P, NT], f32, name="den_pp")
                nc.gpsimd.dma_start(
                    out=den_pp[:],
                    in_=rscratch[0].rearrange("(ic p) -> p ic", p=P),
                )
                recip_pp = rcp.tile([P, NT], f32, name="recip_pp")
                nc.vector.reciprocal(out=recip_pp[:], in_=den_pp[:])

                # ---- numerator numT = (E v)^T : [d, i] (unnormalized) ----
                numT = ohp.tile([P, DC, N], bf16, name="numT")
                for ec in range(DC):
                    pn = [mmps.tile([P, FD], f32, name="mm") for _ in range(IH)]
                    for jc in range(NT):
                        for ih in range(IH):
                            nc.tensor.matmul(
                                pn[ih][:],
                                lhsT=vN[:, jc, ec * P:(ec + 1) * P],
                                rhs=et[:, jc, ih * FD:(ih + 1) * FD],
                                start=(jc == 0),
                                stop=(jc == NT - 1),
                            )
                    for ih in range(IH):
                        nc.scalar.copy(out=numT[:, ec, ih * FD:(ih + 1) * FD], in_=pn[ih][:])

                # ---- output projection via G; normalize per-row (i on
                #      partitions) and accumulate over heads in SBUF ----
                last = b == BPC - 1 and h == H - 1
                for ic in range(NT):
                    # true kernel tail: borrow the now-idle mm-psum slots so
                    # all 8 rep groups run before the recip chain lands
                    if last and ic >= NT - 4:
                        pr = mmps.tile([P, FD], f32, name="mm")
                    else:
                        pr = repps.tile([P, FD], f32, name="pr")
                    for ec in range(DC):
                        nc.tensor.matmul(
                            pr[:],
                            lhsT=numT[:, ec, ic * P:(ic + 1) * P],
                            rhs=gh[:, ec, :],
                            start=(ec == 0),
                            stop=(ec == DC - 1),
                        )
                    nc.vector.scalar_tensor_tensor(
                        out=rep[:, ic, :],
                        in0=pr[:],
                        scalar=recip_pp[:, ic:ic + 1],
                        in1=cv_full[:] if h == 0 else rep[:, ic, :],
                        op0=mybir.AluOpType.mult,
                        op1=mybir.AluOpType.add,
                    )
                    if h == H - 1:
                        eng = nc.scalar if last else nc.sync
                        eng.dma_start(
                            out=out_d[b, ic * P:(ic + 1) * P, :], in_=rep[:, ic, :]
                        )

    nc.finalize()
    return nc


def _prep(k, v, q, Wk, bk, Wv, bv, Wq, bq, Wo, bo):
    """Host-side fusion + layout prep shared by all cores."""
    k, v, q, Wk, bk, Wv, bv, Wq, bq, Wo, bo = (
        np.asarray(x, dtype=np.float32)
        for x in (k, v, q, Wk, bk, Wv, bv, Wq, bq, Wo, bo)
    )
    s = np.float32(D ** -0.5)

    def arr_x(x):  # [B?, D, N] -> [B?, P, DC, N]  (d = dc*P + p)
        b = x.shape[0]
        n = x.shape[2]
        return np.ascontiguousarray(
            x.reshape(b, DC, P, n).transpose(0, 2, 1, 3)
        ).astype(BF16)

    qT = arr_x(q.transpose(0, 2, 1))                    # [BPC*, P, DC, N]
    kT = arr_x(k.transpose(0, 2, 1))
    vN = np.ascontiguousarray(
        v.reshape(B, NT, P, D).transpose(0, 2, 1, 3)
    ).astype(BF16)                                      # [B, P, NT, D] (j on partitions)

    WoR = Wo.reshape(D, D, H)                           # [o, e, h]
    # M = Wq^T Wk / sqrt(D): [h, d, d2];  G^T = Wv^T Wo_h^T: [h, d, o]
    M = np.einsum("hed,heg->hdg", Wq, Wk) * s
    G = np.einsum("hed,oeh->hdo", Wv, WoR)
    Mp = arr_x(M)                                       # [H, P, DC, D]
    Gp = arr_x(G)

    # c[b,h,j] = ((bq Wk) . k[b,j]) / sqrt(D) -> [B, H, P, NT] (j = jc*P + p)
    u = np.einsum("he,hed->hd", bq, Wk)
    c = np.einsum("hd,bjd->bhj", u, k) * s
    cj = np.ascontiguousarray(
        c.reshape(B, H, NT, P).transpose(0, 1, 3, 2)
    ).astype(np.float32)

    cv = (bo + np.einsum("oeh,he->o", WoR, bv)).astype(np.float32).reshape(1, D)
    return qT, kT, vN, Mp, Gp, cj, cv


def kernel(k, v, q, Wk, bk, Wv, bv, Wq, bq, Wo, bo):
    from concourse import bass_utils

    if "nc" not in _cached:
        _cached["nc"] = _build()
    nc = _cached["nc"]

    qT, kT, vN, Mp, Gp, cj, cv = _prep(k, v, q, Wk, bk, Wv, bv, Wq, bq, Wo, bo)

    in_maps = []
    for c in range(NCORES):
        sl = slice(c * BPC, (c + 1) * BPC)
        in_maps.append(
            {
                "qT": qT[sl],
                "kT": kT[sl],
                "vN": vN[sl],
                "Mp": Mp,
                "Gp": Gp,
                "cj": cj[sl],
                "cv": cv,
            }
        )

    res = bass_utils.run_bass_kernel_spmd(nc, in_maps, core_ids=list(range(NCORES)))
    out = np.concatenate([r["out"] for r in res.results], axis=0)
    return out.astype(np.float32)


# revision 37
# speedup vs baseline: 1.0036x; 1.0036x over previous
"""Multi-head attention (per-head full-dim projections) on 8 TRN2 NeuronCores.

Problem: B=16, N=1024, D=512, H=8
  k_/v_/q_ = x @ W{k,v,q}[h].T + b  -> per-head [B,N,D]
  attn = softmax((q_ @ k_^T)/sqrt(D)); out = attn @ v_
  rep = interleave_heads(out) @ Wo.T + bo

Sharding: data parallel over batch (2 batches/core, no collectives).

Fused-projection algebra (host-precomputed, cuts device matmuls 25%):
  scores = (q Wq^T)(k Wk^T)^T/sqrt(D) + bias terms
         = q M k^T + c[j] + (i-only terms that cancel in softmax)
    with M = Wq^T Wk / sqrt(D)  and  c[b,h,j] = ((bq Wk) . k[b,j]) / sqrt(D)
    -> the k-projection disappears; c enters free as the per-partition bias
       of the exp eviction.
  attn @ (v Wv^T + bv) @ Wo_h^T = attn v G^T + bv Wo_h^T
    with G^T = Wv^T Wo_h^T  -> the v-projection disappears; the bv term is a
    constant vector folded into cv = bo + sum_h bv Wo_h^T (host).

Device per (b, h) — all matmuls contract over the partition dim, layouts
arranged on host so there are zero on-device transposes:
  tT[d2,i]  = (lhsT=M tile [d,d2c], rhs=qT [d,i])          32 MM
  S_T[j,i]  = (lhsT=kT [d2,jc],    rhs=tT [d2,i])          64 MM
  E_T = exp(S_T + c[j])  (ACT, per-partition bias; no max-subtract: scores
        ~N(0,1) so exp is safe)
  den[i]: DVE adds over j-chunks + gpsimd partition_all_reduce (no PE);
        reciprocal taken after a tiny DRAM-roundtrip reshape to [i%P, i//P]
  numT[d,i] = (lhsT=v tile [j,dc],  rhs=E_T [j,i])         64 MM
  rep[i,o] += (lhsT=numT [d,ic],    rhs=G^T [d,o])         32 MM
        per-head 1/den[i] fused into the eviction as a per-partition scalar
        (scalar_tensor_tensor: rep = psum*recip + rep; +cv at h==0)
"""

import sys

sys.path.insert(0, "/opt/trn_rl_repo")

from contextlib import ExitStack

import numpy as np
import ml_dtypes

B, N, D, H = 16, 1024, 512, 8
NCORES = 8
BPC = B // NCORES  # batches per core
P = 128
DC = D // P        # 4 feature chunks
NT = N // P        # 8 token chunks
FD = 512           # matmul moving free dim / PSUM bank
IH = N // FD       # 2 halves of the token axis

BF16 = ml_dtypes.bfloat16

_cached = {}


def _build():
    import concourse.bass as bass
    import concourse.tile as tile
    from concourse import bacc, mybir, bass_isa

    f32 = mybir.dt.float32
    bf16 = mybir.dt.bfloat16

    nc = bacc.Bacc(None, target_bir_lowering=False, debug=False)

    qT_d = nc.dram_tensor("qT", [BPC, P, DC, N], bf16, kind="ExternalInput")
    kT_d = nc.dram_tensor("kT", [BPC, P, DC, N], bf16, kind="ExternalInput")
    vN_d = nc.dram_tensor("vN", [BPC, P, NT, D], bf16, kind="ExternalInput")
    m_d = nc.dram_tensor("Mp", [H, P, DC, D], bf16, kind="ExternalInput")
    g_d = nc.dram_tensor("Gp", [H, P, DC, D], bf16, kind="ExternalInput")
    cj_d = nc.dram_tensor("cj", [BPC, H, P, NT], f32, kind="ExternalInput")
    cv_d = nc.dram_tensor("cv", [1, D], f32, kind="ExternalInput")
    out_d = nc.dram_tensor("out", [BPC, N, D], f32, kind="ExternalOutput")

    with tile.TileContext(nc) as tc, ExitStack() as ctx:
        consts = ctx.enter_context(tc.tile_pool(name="consts", bufs=1))
        acts = ctx.enter_context(tc.tile_pool(name="acts", bufs=2))
        wpool = ctx.enter_context(tc.tile_pool(name="wpool", bufs=2))
        projp = ctx.enter_context(tc.tile_pool(name="projp", bufs=2))
        etp = ctx.enter_context(tc.tile_pool(name="etp", bufs=2))
        ohp = ctx.enter_context(tc.tile_pool(name="ohp", bufs=2))
        rcp = ctx.enter_context(tc.tile_pool(name="rcp", bufs=2))
        repp = ctx.enter_context(tc.tile_pool(name="repp", bufs=1))
        cjp = ctx.enter_context(tc.tile_pool(name="cjp", bufs=2))
        mmps = ctx.enter_context(tc.tile_pool(name="mmps", bufs=4, space="PSUM"))
        repps = ctx.enter_context(tc.tile_pool(name="repps", bufs=4, space="PSUM"))
        dramp = ctx.enter_context(tc.tile_pool(name="dramp", bufs=2, space="DRAM"))

        cv_full = consts.tile([P, D], f32)
        nc.gpsimd.dma_start(out=cv_full[:], in_=cv_d[0:1, :].to_broadcast([P, D]))

        for b in range(BPC):
            def load_weights(h):
                mh = wpool.tile([P, DC, D], bf16, name="mh")
                gh = wpool.tile([P, DC, D], bf16, name="gh")
                nc.sync.dma_start(out=mh[:], in_=m_d[h])
                nc.sync.dma_start(out=gh[:], in_=g_d[h])
                return mh, gh

            qT = acts.tile([P, DC, N], bf16, name="qT_sb")
            kT = acts.tile([P, DC, N], bf16, name="kT_sb")
            vN = acts.tile([P, NT, D], bf16, name="vN_sb")
            cj_sb = cjp.tile([P, H, NT], f32, name="cj_sb")
            nc.gpsimd.dma_start(out=cj_sb[:], in_=cj_d[b].rearrange("h p t -> p h t"))
            # interleave weight-h0 and activation chunk DMAs so the first
            # matmuls (Mh dc0 + qT dc0) can start ASAP
            w0 = None
            for dc in range(DC):
                if b == 0 and dc == 0:
                    nc.sync.dma_start(out=qT[:, 0, :], in_=qT_d[b, :, 0, :])
                    w0 = load_weights(0)
                    nc.sync.dma_start(out=kT[:, 0, :], in_=kT_d[b, :, 0, :])
                else:
                    nc.sync.dma_start(out=qT[:, dc, :], in_=qT_d[b, :, dc, :])
                    nc.sync.dma_start(out=kT[:, dc, :], in_=kT_d[b, :, dc, :])
                nc.sync.dma_start(out=vN[:, 2 * dc, :], in_=vN_d[b, :, 2 * dc, :])
                nc.sync.dma_start(out=vN[:, 2 * dc + 1, :], in_=vN_d[b, :, 2 * dc + 1, :])

            rep = repp.tile([P, NT, D], f32, name="rep")

            for h in range(H):
                if b == 0 and h == 0:
                    mh, gh = w0
                else:
                    mh, gh = load_weights(h)

                # ---- tT = (q M)^T : [d2, i] ----
                tT = projp.tile([P, DC, N], bf16, name="tT")
                for ec in range(DC):
                    pq = [mmps.tile([P, FD], f32, name="mm") for _ in range(IH)]
                    for dc in range(DC):
                        for ih in range(IH):
                            nc.tensor.matmul(
                                pq[ih][:],
                                lhsT=mh[:, dc, ec * P:(ec + 1) * P],
                                rhs=qT[:, dc, ih * FD:(ih + 1) * FD],
                                start=(dc == 0),
                                stop=(dc == DC - 1),
                            )
                    for ih in range(IH):
                        nc.scalar.copy(out=tT[:, ec, ih * FD:(ih + 1) * FD], in_=pq[ih][:])

                # ---- scores S_T = (t k^T)^T + c[j]; exp; den on DVE ----
                et = etp.tile([P, NT, N], bf16, name="et")  # E_T [j, i]
                den_acc = rcp.tile([P, N], bf16, name="den_acc")
                den_red = rcp.tile([P, N], f32, name="den_red")
                for jc in range(NT):
                    ps = [mmps.tile([P, FD], f32, name="mm") for _ in range(IH)]
                    for ec in range(DC):
                        for ih in range(IH):
                            nc.tensor.matmul(
                                ps[ih][:],
                                lhsT=kT[:, ec, jc * P:(jc + 1) * P],
                                rhs=tT[:, ec, ih * FD:(ih + 1) * FD],
                                start=(ec == 0),
                                stop=(ec == DC - 1),
                            )
                    for ih in range(IH):
                        nc.scalar.activation(
                            out=et[:, jc, ih * FD:(ih + 1) * FD],
                            in_=ps[ih][:],
                            func=mybir.ActivationFunctionType.Exp,
                            bias=cj_sb[:, h, jc:jc + 1],
                        )
                    if jc == 1:
                        nc.vector.tensor_add(den_acc[:], et[:, 0, :], et[:, 1, :])
                    elif jc > 1:
                        nc.vector.tensor_add(den_acc[:], et[:, jc, :], den_acc[:])

                # ---- partition-reduce den on gpsimd, recip -> [i%P, i//P] ----
                nc.gpsimd.partition_all_reduce(
                    den_red[:], den_acc[:], P, bass_isa.ReduceOp.add
                )
                rscratch = dramp.tile([1, N], f32, name="rscratch")
                nc.gpsimd.dma_start(out=rscratch[:], in_=den_red[0:1, :])
                den_pp = rcp.tile([P, NT], f32, name="den_pp")
                nc.gpsimd.dma_start(
                    out=den_pp[:],
                    in_=rscratch[0].rearrange("(ic p) -> p ic", p=P),
                )
                recip_pp = rcp.tile([P, NT], f32, name="recip_pp")
                nc.vector.reciprocal(out=recip_pp[:], in_=den_pp[:])

                # ---- numerator numT = (E v)^T : [d, i] (unnormalized) ----
                numT = ohp.tile([P, DC, N], bf16, name="numT")
                for ec in range(DC):
                    pn = [mmps.tile([P, FD], f32, name="mm") for _ in range(IH)]
                    for jc in range(NT):
                        for ih in range(IH):
                            nc.tensor.matmul(
                                pn[ih][:],
                                lhsT=vN[:, jc, ec * P:(ec + 1) * P],
                                rhs=et[:, jc, ih * FD:(ih + 1) * FD],
                                start=(jc == 0),
                                stop=(jc == NT - 1),
                            )
                    for ih in range(IH):
                        nc.scalar.copy(out=numT[:, ec, ih * FD:(ih + 1) * FD], in_=pn[ih][:])

                # ---- output projection via G; normalize per-row (i on
                #      partitions) and accumulate over heads in SBUF ----
                for ic in range(NT):
                    pr = repps.tile([P, FD], f32, name="pr")
                    for ec in range(DC):
                        nc.tensor.matmul(
                            pr[:],
                            lhsT=numT[:, ec, ic * P:(ic + 1) * P],
                            rhs=gh[:, ec, :],
                            start=(ec == 0),
                            stop=(ec == DC - 1),
                        )
                    nc.vector.scalar_tensor_tensor(
                        out=rep[:, ic, :],
                        in0=pr[:],
                        scalar=recip_pp[:, ic:ic + 1],
                        in1=cv_full[:] if h == 0 else rep[:, ic, :],
                        op0=mybir.AluOpType.mult,
                        op1=mybir.AluOpType.add,
                    )
                    if h == H - 1:
                        nc.sync.dma_start(
                            out=out_d[b, ic * P:(ic + 1) * P, :], in_=rep[:, ic, :]
                        )

    nc.finalize()
    return nc


def _prep(k, v, q, Wk, bk, Wv, bv, Wq, bq, Wo, bo):
    """Host-side fusion + layout prep shared by all cores."""
    k, v, q, Wk, bk, Wv, bv, Wq, bq, Wo, bo = (
        np.asarray(x, dtype=np.float32)
        for x in (k, v, q, Wk, bk, Wv, bv, Wq, bq, Wo, bo)
    )
    s = np.float32(D ** -0.5)

    def arr_x(x):  # [B?, D, N] -> [B?, P, DC, N]  (d = dc*P + p)
        b = x.shape[0]
        n = x.shape[2]
        return np.ascontiguousarray(
            x.reshape(b, DC, P, n).transpose(0, 2, 1, 3)
        ).astype(BF16)

    qT = arr_x(q.transpose(0, 2, 1))                    # [BPC*, P, DC, N]
    kT = arr_x(k.transpose(0, 2, 1))
    vN = np.ascontiguousarray(
        v.reshape(B, NT, P, D).transpose(0, 2, 1, 3)
    ).astype(BF16)                                      # [B, P, NT, D] (j on partitions)

    WoR = Wo.reshape(D, D, H)                           # [o, e, h]
    # M = Wq^T Wk / sqrt(D): [h, d, d2];  G^T = Wv^T Wo_h^T: [h, d, o]
    M = np.einsum("hed,heg->hdg", Wq, Wk) * s
    G = np.einsum("hed,oeh->hdo", Wv, WoR)
    Mp = arr_x(M)                                       # [H, P, DC, D]
    Gp = arr_x(G)

    # c[b,h,j] = ((bq Wk) . k[b,j]) / sqrt(D) -> [B, H, P, NT] (j = jc*P + p)
    u = np.einsum("he,hed->hd", bq, Wk)
    c = np.einsum("hd,bjd->bhj", u, k) * s
    cj = np.ascontiguousarray(
        c.reshape(B, H, NT, P).transpose(0, 1, 3, 2)
    ).astype(np.float32)

    cv = (bo + np.einsum("oeh,he->o", WoR, bv)).astype(np.float32).reshape(1, D)
    return qT, kT, vN, Mp, Gp, cj, cv


def kernel(k, v, q, Wk, bk, Wv, bv, Wq, bq, Wo, bo):
    from concourse import bass_utils

    if "nc" not in _cached:
        _cached["nc"] = _build()
    nc = _cached["nc"]

    qT, kT, vN, Mp, Gp, cj, cv = _prep(k, v, q, Wk, bk, Wv, bv, Wq, bq, Wo, bo)

    in_maps = []
    for c in range(NCORES):
        sl = slice(c * BPC, (c + 1) * BPC)
        in_maps.append(
            {
                "qT": qT[sl],
                "kT": kT[sl],
                "vN": vN[sl],
                "Mp": Mp,
                "Gp": Gp,
                "cj": cj[sl],
                "cv": cv,
            }
        )

    res = bass_utils.run_bass_kernel_spmd(nc, in_maps, core_ids=list(range(NCORES)))
    out = np.concatenate([r["out"] for r in res.results], axis=0)
    return out.astype(np.float32)
